# revision 12
# baseline (speedup 1.0000x reference)
"""Trainium2 Bass kernel for nn_ExpandingLinear.

Reference computation (B=8192, F0=2048, E1=E2=256, O=1024, F2=2560):
    h1 = concat([x, relu(x[:, e1_parent] * e1_w)], 1)          # [B, 2304]
    h2 = concat([h1, relu(h1[:, e2_parent] * e2_w)], 1)        # [B, 2560]
    W  = scatter_add(zeros(O, F2), (w_rows, w_cols), w_vals)
    b  = scatter_add(zeros(O,), b_idx, b_vals)
    out = h2 @ W.T + b  # [B, O]

Algebraic reduction done on the host (weights only):
    relu(x * w) == w * relu(sign(w) * x) for scalar w, so every embed output
    column is (nonneg scalar) * relu(s * x[:, c]) for some source column c and
    sign s.  Folding each embed column's contribution through W gives

        out = x @ W0t + relu(S ⊙ xg) @ A + 1·bias

    where W0t = W[:, :2048].T, xg = the <=511 distinct (c, s) source columns,
    A is a small host-folded matrix, and an all-ones lhsT row adds the bias.

Device kernel (SPMD over 8 cores, batch-sharded 1024 rows/core):
    - x part: 16 bf16 k-tiles [128, 2048] streamed as one DRAM "slab" per
      core laid out [wc_n0 (512) | lhsT source (1024) | wc_n1 (512)]; pass A
      streams the 384 KiB col-0:1536 chunk per k-tile, pass B the 128 KiB
      tail.  k-tile 0's pass-A region is split into two dedicated tiles
      leading the sync / scalar queues (tx: cols 0:640, ty: cols 640:1536)
      so the first matmul's data dep is precise and its receipt is first in
      line (first-DMA receipt latency is ~5us on every queue).
    - relu part: all pairs kept, padded to 512 rows (incl. the bias row),
      and run in FP8 e4m3 DoubleRow perf mode: one DoubleRow matmul
      contracts 256 rows (2 fp8 values per PE cell) at 2x rate, so the
      whole block is 2 matmuls of 256 cycles per (m-group, n-half) vs 4
      bf16 k-tiles at 512 each — PE drops from 20 to ~17 k-tile
      equivalents.  Scales: xg/32 (folded into the sign scalars used by the
      on-device relu) and A*32 (host), so the product needs no rescale and
      accumulates into the same PSUM chain as the bf16 x part.  Measured
      max rel err ~= 1.4e-2 (allowed 2e-2; bf16-everything baseline 4e-3).
    - PE warm-up matmuls run on a MEMSET tile (no DMA dependency) so the
      HAM clock gate flips up during the DMA-latency window, bridging
      until the first chunk's receipt (an idle PE gap before the stream
      delays the flip by several us).
    - two passes over all 8 m-groups with full-K PSUM accumulation
      (18 matmuls per group, start/stop once): pass A (n=0) k-major to
      consume k-tiles in DMA arrival order, pass B (n=1) group-major so
      drains+stores pipeline into the tail; the last group is split
      N=384+128 so only a [128,128] cast + 32 KiB store trail the final
      matmul.
    - outputs stored as bf16 (host converts to fp32).
"""

import numpy as np

import concourse.bass as bass
import concourse.tile as tile
from concourse import bacc, mybir
from concourse.bass_utils import run_bass_kernel_spmd

B, F0, E1, E2, O = 8192, 2048, 256, 256, 1024
F1 = F0 + E1
F2 = F1 + E2
N_CORES = 8
BS = B // N_CORES          # 1024 batch rows per core
P = 128                    # partitions
KT_X = F0 // P             # 16 k-tiles of raw x
N_HALF = 512               # matmul moving free dim
MT = BS // P               # 8 m-groups of 128 batch rows
MDT = mybir.dt.bfloat16
F8 = mybir.dt.float8e4     # ml_dtypes.float8_e4m3
SLAB_W = N_HALF + BS + N_HALF   # 2048: [wc_n0 | g | wc_n1]
G0, G1 = N_HALF, N_HALF + BS    # g columns 512:1536
C1 = N_HALF + BS                # pass-A chunk covers cols 0:1536
TX_W = N_HALF + P               # 640: first-matmul dependency (wc_n0 + g_m0)
NR = 512                        # relu block rows (pairs + bias + padding)
RB = NR // P                    # 4 row-blocks
NQ = RB // 2                    # 2 DoubleRow matmuls cover the block
SX = 1.0 / 32.0                 # fp8 scale on relu activations (1/SX on A)
N_WARM = 15                     # warm-up matmuls (N=256) bridging to the stream

_CACHE = {}


def _fold_weights(e1_w, e2_w, w_vals, b_vals, e1_parent, e2_parent,
                  w_rows, w_cols, b_idx):
    """Host-side weight preprocessing: densify W/b and fold the two embed
    layers' contributions into (cols, signs, A) so the device computes
    out = x @ W0t + relu(sign*x[:, cols]) @ A + bias."""
    W = np.bincount(w_rows.astype(np.int64) * F2 + w_cols.astype(np.int64),
                    weights=w_vals.astype(np.float64),
                    minlength=O * F2).reshape(O, F2)
    bias = np.bincount(b_idx.astype(np.int64), weights=b_vals.astype(np.float64),
                       minlength=O)
    W0t = W[:, :F0].T.copy()   # [2048, 1024]
    W1t = W[:, F0:F1].T        # [256, 1024]  layer-1 embed rows
    W2t = W[:, F1:F2].T        # [256, 1024]  layer-2 embed rows

    # each embed column j contributes scale*relu(s*x[:, c]) with weight row w
    # accumulate per (c, s): A_map[(c, s)] += scale * w_row
    A_map = {}

    def acc(c, s, scale, wrow):
        if scale == 0.0:
            return
        key = (int(c), int(s))
        if key in A_map:
            A_map[key] = A_map[key] + scale * wrow
        else:
            A_map[key] = scale * wrow

    e1_parent = e1_parent.astype(np.int64)
    e2_parent = e2_parent.astype(np.int64)
    e1_w64 = e1_w.astype(np.float64)
    e2_w64 = e2_w.astype(np.float64)

    for j in range(E1):
        w = e1_w64[j]
        s = 1 if w >= 0 else -1
        acc(e1_parent[j], s, abs(w), W1t[j])
    for j in range(E2):
        q = e2_parent[j]
        w = e2_w64[j]
        if q < F0:
            s = 1 if w >= 0 else -1
            acc(q, s, abs(w), W2t[j])
        else:
            # refers to layer-1 embed column m1: h1e[:, m1] >= 0 always
            if w < 0:
                continue  # relu(negative * nonneg) == 0
            m1 = q - F0
            w1 = e1_w64[m1]
            s = 1 if w1 >= 0 else -1
            acc(e1_parent[m1], s, w * abs(w1), W2t[j])

    pairs = sorted(A_map.keys())
    n_pairs = len(pairs)
    assert n_pairs <= NR - 1, n_pairs
    cols = np.zeros(NR, dtype=np.int64)
    signs = np.ones(NR, dtype=np.float32)
    A = np.zeros((NR, O), dtype=np.float64)
    for i, (c, s) in enumerate(pairs):
        cols[i] = c
        signs[i] = s
        A[i] = A_map[(c, s)]
    A[NR - 1] = bias    # bias row rides the all-ones relu column
    return W0t.astype(np.float32), A.astype(np.float32), cols, signs


def _build_program():
    """Build + compile the SPMD Bass program (same for every core)."""
    nc = bacc.Bacc("TRN2", target_bir_lowering=False, debug=False,
                   num_devices=N_CORES)

    slab_d = nc.dram_tensor("slab", [KT_X, P, SLAB_W], MDT,
                            kind="ExternalInput")
    rs_d = nc.dram_tensor("rs", [P, RB * BS], MDT, kind="ExternalInput")
    w8_d = nc.dram_tensor("w8", [P, RB * O], F8, kind="ExternalInput")
    sg_d = nc.dram_tensor("sg", [P, RB], mybir.dt.float32,
                          kind="ExternalInput")
    out_d = nc.dram_tensor("out", [MT, 2, P, N_HALF], MDT,
                           kind="ExternalOutput")

    with tile.TileContext(nc) as tc:
        with (
            tc.tile_pool(name="sbuf", bufs=1) as pool,
            tc.tile_pool(name="psum", bufs=8, space="PSUM") as psum,
        ):
            # k-tile 0's pass-A region as two dedicated tiles with precise
            # tile-level deps: tx (wc_n0 + g_m0) leads the sync queue so the
            # first matmul can go at its receipt; ty (g_m1..m7) leads the
            # scalar queue so it lands in parallel.  (The GpSimd DMA queue
            # is NOT used: measured ~1.5us slower to first receipt.)
            txt = pool.tile([P, TX_W], MDT, tag="tx", name="tx")
            tyt = pool.tile([P, C1 - TX_W], MDT, tag="ty", name="ty")
            nc.sync.dma_start(txt[:], slab_d[0][:, :TX_W],
                              single_packet=True)
            nc.scalar.dma_start(tyt[:], slab_d[0][:, TX_W:C1],
                                single_packet=True)

            # PE warm-up on a memset tile: no DMA dependency, so the HAM
            # clock gate flips up while the input stream is still in
            # flight (cold matmuls run well below boost clock)
            wz = pool.tile([P, 256], MDT, tag="wz", name="wz")
            nc.vector.memset(wz[:], 0.0)
            wps = psum.tile([P, N_HALF], mybir.dt.float32, tag="ps",
                            name="wps")
            for _ in range(N_WARM):
                nc.tensor.matmul(wps[:, :256], wz[:, :P], wz[:],
                                 start=True, stop=True)

            sg_sb = pool.tile([P, RB], mybir.dt.float32, tag="sg")
            slabs = [None] + [
                pool.tile([P, SLAB_W], MDT, tag=f"s{kt}", name=f"s{kt}")
                for kt in range(1, KT_X)]
            s0b = pool.tile([P, N_HALF], MDT, tag="s0b", name="s0b")
            rst = pool.tile([P, RB, BS], MDT, tag="rs", name="rs")
            w8t = pool.tile([P, RB, O], F8, tag="w8", name="w8")
            # main input stream on the in-order sync queue, in consumption
            # order; sg rides the scalar queue
            nc.scalar.dma_start(sg_sb[:], sg_d[:])
            for kt in range(1, KT_X):
                nc.sync.dma_start(slabs[kt][:, :C1], slab_d[kt][:, :C1])
            # relu-block sources + fp8 weights: consumed at the tail of
            # every pass-A chain, long after k-tile 15 lands
            nc.sync.dma_start(rst[:], rs_d[:])
            nc.sync.dma_start(w8t[:], w8_d[:])
            # pass-B chunks (wc_n1) after the whole pass-A stream
            nc.sync.dma_start(s0b[:], slab_d[0][:, C1:])
            for kt in range(1, KT_X):
                nc.sync.dma_start(slabs[kt][:, C1:], slab_d[kt][:, C1:])

            # fp8 sign-relu blocks: r2[b] = fp8(max(sg_b * rs_b, 0));
            # sg_b = sign/32, so products with the host-side A*32 need no
            # rescale.  The all-ones bias column becomes 1/32 exactly.
            r2t = pool.tile([P, RB, BS], F8, tag="r2", name="r2")
            for b in range(RB):
                nc.vector.tensor_scalar(r2t[:, b], rst[:, b],
                                        sg_sb[:, b:b + 1], 0.0,
                                        mybir.AluOpType.mult,
                                        mybir.AluOpType.max)

            def lh(kt, m):
                if kt == 0:
                    if m == 0:
                        return txt[:, N_HALF:TX_W]
                    return tyt[:, (m - 1) * P:m * P]
                return slabs[kt][:, G0 + m * P:G0 + (m + 1) * P]

            def fp8_mms(ps, m, c0, c1, last_eng=None):
                """The relu block as NQ DoubleRow fp8 matmuls, closing the
                accumulation chain over output cols [c0:c1)."""
                for q in range(NQ):
                    nc.tensor.matmul(
                        ps[:], r2t[:, 2 * q:2 * q + 2, m * P:(m + 1) * P],
                        w8t[:, 2 * q:2 * q + 2, c0:c1],
                        start=False, stop=(q == NQ - 1),
                        perf_mode=mybir.MatmulPerfMode.DoubleRow)

            # pass A (n=0): k-major so the PE consumes k-tiles in DMA
            # arrival order; each m-group accumulates 16 bf16 k-tiles plus
            # the fp8 relu block in its PSUM bank, drains once, stores bf16
            psA = [psum.tile([P, N_HALF], mybir.dt.float32, tag="ps",
                             name=f"pA{m}") for m in range(MT)]
            for kt in range(KT_X):
                for m in range(MT):
                    rhs = txt[:, :N_HALF] if kt == 0 else slabs[kt][:, :N_HALF]
                    nc.tensor.matmul(psA[m][:], lh(kt, m), rhs,
                                     start=(kt == 0), stop=False)
            for m in range(MT):
                fp8_mms(psA[m], m, 0, N_HALF)
                oA = pool.tile([P, N_HALF], MDT, tag=f"oA{m}")
                nc.vector.tensor_copy(oA[:], psA[m][:])
                nc.scalar.dma_start(out_d[m][0], oA[:])

            # pass B (n=1): group-major so each group's drain + store
            # pipelines right behind its last matmul (everything resident)
            for m in range(MT - 1):
                psB = psum.tile([P, N_HALF], mybir.dt.float32, tag="ps",
                                name=f"pB{m}")
                for kt in range(KT_X):
                    rhs = s0b[:] if kt == 0 else slabs[kt][:, C1:]
                    nc.tensor.matmul(psB[:], lh(kt, m), rhs,
                                     start=(kt == 0), stop=False)
                fp8_mms(psB, m, N_HALF, 2 * N_HALF)
                oB = pool.tile([P, N_HALF], MDT, tag=f"oB{m}")
                nc.vector.tensor_copy(oB[:], psB[:])
                eng = nc.sync if m % 2 else nc.scalar
                eng.dma_start(out_d[m][1], oB[:])

            # the very last group (m=7) runs as an N=384 chain then an
            # N=128 chain: chain a's drain + 96 KiB store hide under chain
            # b's matmuls, so only a [128,128] cast + 32 KiB store +
            # receipt sit after the final matmul
            m = MT - 1
            NA = 384
            for h, (c0, c1) in enumerate(((0, NA), (NA, N_HALF))):
                psB = psum.tile([P, c1 - c0], mybir.dt.float32, tag="ps",
                                name=f"pB7{h}")
                for kt in range(KT_X):
                    rhs = (s0b[:, c0:c1] if kt == 0
                           else slabs[kt][:, C1 + c0:C1 + c1])
                    nc.tensor.matmul(psB[:], lh(kt, m), rhs,
                                     start=(kt == 0), stop=False)
                fp8_mms(psB, m, N_HALF + c0, N_HALF + c1)
                oB = pool.tile([P, c1 - c0], MDT, tag=f"oB7{h}")
                nc.vector.tensor_copy(oB[:], psB[:])
                eng = nc.scalar if h == 0 else nc.sync
                eng.dma_start(out_d[m][1][:, c0:c1], oB[:],
                              single_packet=(h == 1))

    nc.compile()
    return nc


def _prepare(input, e1_w, e2_w, w_vals, b_vals, e1_parent, e2_parent,
             w_rows, w_cols, b_idx):
    """Host prep shared by kernel() and the profiling harness: returns
    (nc, in_maps)."""
    import ml_dtypes
    bf = np.dtype(ml_dtypes.bfloat16)
    f8 = np.dtype(ml_dtypes.float8_e4m3)
    x = np.asarray(input, dtype=np.float32)
    W0t, A, cols, signs = _fold_weights(
        np.asarray(e1_w), np.asarray(e2_w), np.asarray(w_vals),
        np.asarray(b_vals), np.asarray(e1_parent), np.asarray(e2_parent),
        np.asarray(w_rows), np.asarray(w_cols), np.asarray(b_idx))

    # x-part weight slab rows (bf16) and DoubleRow-packed fp8 relu weights
    wc3 = W0t.astype(bf).reshape(KT_X, P, O)
    w8 = np.ascontiguousarray(
        (A * (1.0 / SX)).astype(np.float32).astype(f8)
        .reshape(RB, P, O).transpose(1, 0, 2)).reshape(P, RB * O)
    sg = np.ascontiguousarray(
        (signs * SX).reshape(RB, P).T.astype(np.float32))

    if "prog" not in _CACHE:
        _CACHE["prog"] = _build_program()
    nc = _CACHE["prog"]

    xg_full = x[:, cols]               # [B, 512] gathered source columns
    xg_full[:, NR - 1] = 1.0           # all-ones bias column (sign is +1)
    x16 = x.astype(bf)
    xg16 = xg_full.astype(bf)
    in_maps = []
    for c in range(N_CORES):
        sl = slice(c * BS, (c + 1) * BS)
        g = x16[sl].T.reshape(KT_X, P, BS)
        slab = np.empty((KT_X, P, SLAB_W), dtype=bf)
        slab[:, :, :N_HALF] = wc3[:, :, :N_HALF]
        slab[:, :, G0:G1] = g
        slab[:, :, C1:] = wc3[:, :, N_HALF:]
        rs = np.ascontiguousarray(
            xg16[sl].T.reshape(RB, P, BS).transpose(1, 0, 2)).reshape(
                P, RB * BS)
        in_maps.append({"slab": slab, "rs": rs, "w8": w8, "sg": sg})
    return nc, in_maps


def kernel(input, e1_w, e2_w, w_vals, b_vals, e1_parent, e2_parent,
           w_rows, w_cols, b_idx):
    nc, in_maps = _prepare(input, e1_w, e2_w, w_vals, b_vals,
                           e1_parent, e2_parent, w_rows, w_cols, b_idx)
    res = run_bass_kernel_spmd(nc, in_maps, list(range(N_CORES)))
    out = np.concatenate(
        [np.asarray(res.results[c]["out"]).transpose(0, 2, 1, 3)
         .reshape(BS, O).astype(np.float32)
         for c in range(N_CORES)], axis=0)
    return out


# revision 13
# speedup vs baseline: 1.0092x; 1.0092x over previous
"""Trainium2 Bass kernel for nn_ExpandingLinear.

Reference computation (B=8192, F0=2048, E1=E2=256, O=1024, F2=2560):
    h1 = concat([x, relu(x[:, e1_parent] * e1_w)], 1)          # [B, 2304]
    h2 = concat([h1, relu(h1[:, e2_parent] * e2_w)], 1)        # [B, 2560]
    W  = scatter_add(zeros(O, F2), (w_rows, w_cols), w_vals)
    b  = scatter_add(zeros(O,), b_idx, b_vals)
    out = h2 @ W.T + b  # [B, O]

Algebraic reduction done on the host (weights only):
    relu(x * w) == w * relu(sign(w) * x) for scalar w, so every embed output
    column is (nonneg scalar) * relu(s * x[:, c]) for some source column c and
    sign s.  Folding each embed column's contribution through W gives

        out = x @ W0t + relu(S ⊙ xg) @ A + 1·bias

    where W0t = W[:, :2048].T, xg = the <=511 distinct (c, s) source columns,
    A is a small host-folded matrix, and an all-ones lhsT row adds the bias.

Device kernel (SPMD over 8 cores, batch-sharded 1024 rows/core):
    - x part: 16 bf16 k-tiles [128, 2048] streamed as one DRAM "slab" per
      core laid out [wc_n0 (512) | lhsT source (1024) | wc_n1 (512)]; pass A
      streams the 384 KiB col-0:1536 chunk per k-tile, pass B the 128 KiB
      tail.  k-tile 0's pass-A region is split into two dedicated tiles
      leading the sync / scalar queues (tx: cols 0:640, ty: cols 640:1536)
      so the first matmul's data dep is precise and its receipt is first in
      line (first-DMA receipt latency is ~5us on every queue).
    - relu part: all pairs kept, padded to 512 rows (incl. the bias row),
      and run in FP8 e4m3 DoubleRow perf mode: one DoubleRow matmul
      contracts 256 rows (2 fp8 values per PE cell) at 2x rate, so the
      whole block is 2 matmuls of 256 cycles per (m-group, n-half) vs 4
      bf16 k-tiles at 512 each — PE drops from 20 to ~17 k-tile
      equivalents.  Scales: xg/32 (folded into the sign scalars used by the
      on-device relu) and A*32 (host), so the product needs no rescale and
      accumulates into the same PSUM chain as the bf16 x part.  Measured
      max rel err ~= 1.4e-2 (allowed 2e-2; bf16-everything baseline 4e-3).
    - PE warm-up matmuls run on a MEMSET tile (no DMA dependency) so the
      HAM clock gate flips up during the DMA-latency window, bridging
      until the first chunk's receipt (an idle PE gap before the stream
      delays the flip by several us).
    - two passes over all 8 m-groups with full-K PSUM accumulation
      (18 matmuls per group, start/stop once): pass A (n=0) k-major to
      consume k-tiles in DMA arrival order, pass B (n=1) group-major so
      drains+stores pipeline into the tail; the last group is split
      N=384+128 so only a [128,128] cast + 32 KiB store trail the final
      matmul.
    - outputs stored as bf16 (host converts to fp32).
"""

import numpy as np

import concourse.bass as bass
import concourse.tile as tile
from concourse import bacc, mybir
from concourse.bass_utils import run_bass_kernel_spmd

B, F0, E1, E2, O = 8192, 2048, 256, 256, 1024
F1 = F0 + E1
F2 = F1 + E2
N_CORES = 8
BS = B // N_CORES          # 1024 batch rows per core
P = 128                    # partitions
KT_X = F0 // P             # 16 k-tiles of raw x
N_HALF = 512               # matmul moving free dim
MT = BS // P               # 8 m-groups of 128 batch rows
MDT = mybir.dt.bfloat16
F8 = mybir.dt.float8e4     # ml_dtypes.float8_e4m3
SLAB_W = N_HALF + BS + N_HALF   # 2048: [wc_n0 | g | wc_n1]
G0, G1 = N_HALF, N_HALF + BS    # g columns 512:1536
C1 = N_HALF + BS                # pass-A chunk covers cols 0:1536
TX_W = N_HALF + P               # 640: first-matmul dependency (wc_n0 + g_m0)
NR = 512                        # relu block rows (pairs + bias + padding)
RB = NR // P                    # 4 row-blocks
NQ = RB // 2                    # 2 DoubleRow matmuls cover the block
SX = 1.0 / 32.0                 # fp8 scale on relu activations (1/SX on A)
N_WARM = 15                     # warm-up matmuls (N=256) bridging to the stream

_CACHE = {}


def _fold_weights(e1_w, e2_w, w_vals, b_vals, e1_parent, e2_parent,
                  w_rows, w_cols, b_idx):
    """Host-side weight preprocessing: densify W/b and fold the two embed
    layers' contributions into (cols, signs, A) so the device computes
    out = x @ W0t + relu(sign*x[:, cols]) @ A + bias."""
    W = np.bincount(w_rows.astype(np.int64) * F2 + w_cols.astype(np.int64),
                    weights=w_vals.astype(np.float64),
                    minlength=O * F2).reshape(O, F2)
    bias = np.bincount(b_idx.astype(np.int64), weights=b_vals.astype(np.float64),
                       minlength=O)
    W0t = W[:, :F0].T.copy()   # [2048, 1024]
    W1t = W[:, F0:F1].T        # [256, 1024]  layer-1 embed rows
    W2t = W[:, F1:F2].T        # [256, 1024]  layer-2 embed rows

    # each embed column j contributes scale*relu(s*x[:, c]) with weight row w
    # accumulate per (c, s): A_map[(c, s)] += scale * w_row
    A_map = {}

    def acc(c, s, scale, wrow):
        if scale == 0.0:
            return
        key = (int(c), int(s))
        if key in A_map:
            A_map[key] = A_map[key] + scale * wrow
        else:
            A_map[key] = scale * wrow

    e1_parent = e1_parent.astype(np.int64)
    e2_parent = e2_parent.astype(np.int64)
    e1_w64 = e1_w.astype(np.float64)
    e2_w64 = e2_w.astype(np.float64)

    for j in range(E1):
        w = e1_w64[j]
        s = 1 if w >= 0 else -1
        acc(e1_parent[j], s, abs(w), W1t[j])
    for j in range(E2):
        q = e2_parent[j]
        w = e2_w64[j]
        if q < F0:
            s = 1 if w >= 0 else -1
            acc(q, s, abs(w), W2t[j])
        else:
            # refers to layer-1 embed column m1: h1e[:, m1] >= 0 always
            if w < 0:
                continue  # relu(negative * nonneg) == 0
            m1 = q - F0
            w1 = e1_w64[m1]
            s = 1 if w1 >= 0 else -1
            acc(e1_parent[m1], s, w * abs(w1), W2t[j])

    pairs = sorted(A_map.keys())
    n_pairs = len(pairs)
    assert n_pairs <= NR - 1, n_pairs
    cols = np.zeros(NR, dtype=np.int64)
    signs = np.ones(NR, dtype=np.float32)
    A = np.zeros((NR, O), dtype=np.float64)
    for i, (c, s) in enumerate(pairs):
        cols[i] = c
        signs[i] = s
        A[i] = A_map[(c, s)]
    A[NR - 1] = bias    # bias row rides the all-ones relu column
    return W0t.astype(np.float32), A.astype(np.float32), cols, signs


def _build_program():
    """Build + compile the SPMD Bass program (same for every core)."""
    nc = bacc.Bacc("TRN2", target_bir_lowering=False, debug=False,
                   num_devices=N_CORES)

    slab_d = nc.dram_tensor("slab", [KT_X, P, SLAB_W], MDT,
                            kind="ExternalInput")
    rs_d = nc.dram_tensor("rs", [P, RB * BS], MDT, kind="ExternalInput")
    w8_d = nc.dram_tensor("w8", [P, RB * O], F8, kind="ExternalInput")
    sg_d = nc.dram_tensor("sg", [P, RB], mybir.dt.float32,
                          kind="ExternalInput")
    out_d = nc.dram_tensor("out", [MT, 2, P, N_HALF], MDT,
                           kind="ExternalOutput")

    with tile.TileContext(nc) as tc:
        with (
            tc.tile_pool(name="sbuf", bufs=1) as pool,
            tc.tile_pool(name="psum", bufs=8, space="PSUM") as psum,
        ):
            # k-tile 0's pass-A region as two dedicated tiles with precise
            # tile-level deps: tx (wc_n0 + g_m0) leads the sync queue so the
            # first matmul can go at its receipt; ty (g_m1..m7) leads the
            # scalar queue so it lands in parallel.  (The GpSimd DMA queue
            # is NOT used: measured ~1.5us slower to first receipt.)
            txt = pool.tile([P, TX_W], MDT, tag="tx", name="tx")
            tyt = pool.tile([P, C1 - TX_W], MDT, tag="ty", name="ty")
            nc.sync.dma_start(txt[:], slab_d[0][:, :TX_W])
            nc.scalar.dma_start(tyt[:], slab_d[0][:, TX_W:C1])

            # PE warm-up on a memset tile: no DMA dependency, so the HAM
            # clock gate flips up while the input stream is still in
            # flight (cold matmuls run well below boost clock)
            wz = pool.tile([P, 256], MDT, tag="wz", name="wz")
            nc.vector.memset(wz[:], 0.0)
            wps = psum.tile([P, N_HALF], mybir.dt.float32, tag="ps",
                            name="wps")
            for _ in range(N_WARM):
                nc.tensor.matmul(wps[:, :256], wz[:, :P], wz[:],
                                 start=True, stop=True)

            sg_sb = pool.tile([P, RB], mybir.dt.float32, tag="sg")
            slabs = [None] + [
                pool.tile([P, SLAB_W], MDT, tag=f"s{kt}", name=f"s{kt}")
                for kt in range(1, KT_X)]
            s0b = pool.tile([P, N_HALF], MDT, tag="s0b", name="s0b")
            rst = pool.tile([P, RB, BS], MDT, tag="rs", name="rs")
            w8t = pool.tile([P, RB, O], F8, tag="w8", name="w8")
            # main input stream on the in-order sync queue, in consumption
            # order; sg rides the scalar queue
            nc.scalar.dma_start(sg_sb[:], sg_d[:])
            for kt in range(1, KT_X):
                nc.sync.dma_start(slabs[kt][:, :C1], slab_d[kt][:, :C1])
            # relu-block sources + fp8 weights: consumed at the tail of
            # every pass-A chain, long after k-tile 15 lands
            nc.sync.dma_start(rst[:], rs_d[:])
            nc.sync.dma_start(w8t[:], w8_d[:])
            # pass-B chunks (wc_n1) after the whole pass-A stream
            nc.sync.dma_start(s0b[:], slab_d[0][:, C1:])
            for kt in range(1, KT_X):
                nc.sync.dma_start(slabs[kt][:, C1:], slab_d[kt][:, C1:])

            # fp8 sign-relu blocks: r2[b] = fp8(max(sg_b * rs_b, 0));
            # sg_b = sign/32, so products with the host-side A*32 need no
            # rescale.  The all-ones bias column becomes 1/32 exactly.
            r2t = pool.tile([P, RB, BS], F8, tag="r2", name="r2")
            for b in range(RB):
                nc.vector.tensor_scalar(r2t[:, b], rst[:, b],
                                        sg_sb[:, b:b + 1], 0.0,
                                        mybir.AluOpType.mult,
                                        mybir.AluOpType.max)

            def lh(kt, m):
                if kt == 0:
                    if m == 0:
                        return txt[:, N_HALF:TX_W]
                    return tyt[:, (m - 1) * P:m * P]
                return slabs[kt][:, G0 + m * P:G0 + (m + 1) * P]

            def fp8_mms(ps, m, c0, c1, last_eng=None):
                """The relu block as NQ DoubleRow fp8 matmuls, closing the
                accumulation chain over output cols [c0:c1)."""
                for q in range(NQ):
                    nc.tensor.matmul(
                        ps[:], r2t[:, 2 * q:2 * q + 2, m * P:(m + 1) * P],
                        w8t[:, 2 * q:2 * q + 2, c0:c1],
                        start=False, stop=(q == NQ - 1),
                        perf_mode=mybir.MatmulPerfMode.DoubleRow)

            # pass A (n=0): k-major so the PE consumes k-tiles in DMA
            # arrival order; each m-group accumulates 16 bf16 k-tiles plus
            # the fp8 relu block in its PSUM bank, drains once, stores bf16
            psA = [psum.tile([P, N_HALF], mybir.dt.float32, tag="ps",
                             name=f"pA{m}") for m in range(MT)]
            for kt in range(KT_X):
                for m in range(MT):
                    rhs = txt[:, :N_HALF] if kt == 0 else slabs[kt][:, :N_HALF]
                    nc.tensor.matmul(psA[m][:], lh(kt, m), rhs,
                                     start=(kt == 0), stop=False)
            for m in range(MT):
                fp8_mms(psA[m], m, 0, N_HALF)
                oA = pool.tile([P, N_HALF], MDT, tag=f"oA{m}")
                nc.vector.tensor_copy(oA[:], psA[m][:])
                nc.scalar.dma_start(out_d[m][0], oA[:])

            # pass B (n=1): group-major so each group's drain + store
            # pipelines right behind its last matmul (everything resident)
            for m in range(MT - 1):
                psB = psum.tile([P, N_HALF], mybir.dt.float32, tag="ps",
                                name=f"pB{m}")
                for kt in range(KT_X):
                    rhs = s0b[:] if kt == 0 else slabs[kt][:, C1:]
                    nc.tensor.matmul(psB[:], lh(kt, m), rhs,
                                     start=(kt == 0), stop=False)
                fp8_mms(psB, m, N_HALF, 2 * N_HALF)
                oB = pool.tile([P, N_HALF], MDT, tag=f"oB{m}")
                nc.vector.tensor_copy(oB[:], psB[:])
                eng = nc.sync if m % 2 else nc.scalar
                eng.dma_start(out_d[m][1], oB[:])

            # the very last group (m=7) runs as an N=384 chain then an
            # N=128 chain: chain a's drain + 96 KiB store hide under chain
            # b's matmuls, so only a [128,128] cast + 32 KiB store +
            # receipt sit after the final matmul
            m = MT - 1
            NA = 384
            for h, (c0, c1) in enumerate(((0, NA), (NA, N_HALF))):
                psB = psum.tile([P, c1 - c0], mybir.dt.float32, tag="ps",
                                name=f"pB7{h}")
                for kt in range(KT_X):
                    rhs = (s0b[:, c0:c1] if kt == 0
                           else slabs[kt][:, C1 + c0:C1 + c1])
                    nc.tensor.matmul(psB[:], lh(kt, m), rhs,
                                     start=(kt == 0), stop=False)
                fp8_mms(psB, m, N_HALF + c0, N_HALF + c1)
                oB = pool.tile([P, c1 - c0], MDT, tag=f"oB7{h}")
                nc.vector.tensor_copy(oB[:], psB[:])
                eng = nc.scalar if h == 0 else nc.sync
                eng.dma_start(out_d[m][1][:, c0:c1], oB[:])

    nc.compile()
    return nc


def _prepare(input, e1_w, e2_w, w_vals, b_vals, e1_parent, e2_parent,
             w_rows, w_cols, b_idx):
    """Host prep shared by kernel() and the profiling harness: returns
    (nc, in_maps)."""
    import ml_dtypes
    bf = np.dtype(ml_dtypes.bfloat16)
    f8 = np.dtype(ml_dtypes.float8_e4m3)
    x = np.asarray(input, dtype=np.float32)
    W0t, A, cols, signs = _fold_weights(
        np.asarray(e1_w), np.asarray(e2_w), np.asarray(w_vals),
        np.asarray(b_vals), np.asarray(e1_parent), np.asarray(e2_parent),
        np.asarray(w_rows), np.asarray(w_cols), np.asarray(b_idx))

    # x-part weight slab rows (bf16) and DoubleRow-packed fp8 relu weights
    wc3 = W0t.astype(bf).reshape(KT_X, P, O)
    w8 = np.ascontiguousarray(
        (A * (1.0 / SX)).astype(np.float32).astype(f8)
        .reshape(RB, P, O).transpose(1, 0, 2)).reshape(P, RB * O)
    sg = np.ascontiguousarray(
        (signs * SX).reshape(RB, P).T.astype(np.float32))

    if "prog" not in _CACHE:
        _CACHE["prog"] = _build_program()
    nc = _CACHE["prog"]

    xg_full = x[:, cols]               # [B, 512] gathered source columns
    xg_full[:, NR - 1] = 1.0           # all-ones bias column (sign is +1)
    x16 = x.astype(bf)
    xg16 = xg_full.astype(bf)
    in_maps = []
    for c in range(N_CORES):
        sl = slice(c * BS, (c + 1) * BS)
        g = x16[sl].T.reshape(KT_X, P, BS)
        slab = np.empty((KT_X, P, SLAB_W), dtype=bf)
        slab[:, :, :N_HALF] = wc3[:, :, :N_HALF]
        slab[:, :, G0:G1] = g
        slab[:, :, C1:] = wc3[:, :, N_HALF:]
        rs = np.ascontiguousarray(
            xg16[sl].T.reshape(RB, P, BS).transpose(1, 0, 2)).reshape(
                P, RB * BS)
        in_maps.append({"slab": slab, "rs": rs, "w8": w8, "sg": sg})
    return nc, in_maps


def kernel(input, e1_w, e2_w, w_vals, b_vals, e1_parent, e2_parent,
           w_rows, w_cols, b_idx):
    nc, in_maps = _prepare(input, e1_w, e2_w, w_vals, b_vals,
                           e1_parent, e2_parent, w_rows, w_cols, b_idx)
    res = run_bass_kernel_spmd(nc, in_maps, list(range(N_CORES)))
    out = np.concatenate(
        [np.asarray(res.results[c]["out"]).transpose(0, 2, 1, 3)
         .reshape(BS, O).astype(np.float32)
         for c in range(N_CORES)], axis=0)
    return out
